# revision 1
# baseline (speedup 1.0000x reference)
"""APPNP propagation kernel for Trainium2 (8 NeuronCores, SPMD).

Algorithm (reference):
    out_deg/in_deg from edges; K=10 iterations of
    feat = 0.9 * (segment_sum(h[src], dst) * dst_norm) + 0.1 * feat0,
    with h = feat * src_norm.

Distribution: nodes sharded across 8 cores by destination (12544
nodes/core = 98 blocks of 128). Each iteration:
  1. every core computes h (bf16) for its shard; shards are AllGathered
     into a compact [100352, 48] bf16 table in each core's DRAM, then
     expanded to 256B-stride rows [100352, 128] (dma_gather needs
     256B-multiple rows),
  2. each core gathers the source rows for its dst-sorted edges with
     gpsimd.dma_gather (int16 indices -> 4 src-quarter sub-tables;
     edges are grouped (dst_block, src_quarter) and padded to 128-edge
     tiles; all-pad tiles gather row 0 and are nulled by the selection
     matrix),
  3. segment-sum is a one-hot matmul per 128-edge tile accumulated in
     PSUM per 128-node destination block (selection matrices built
     on-device with iota/is_equal, bf16),
  4. epilogue applies dst_norm, the alpha anchor, and produces next h.

Host-side prep (sharding, degrees, sorting, padding) is pure numpy;
the K-iteration loop runs entirely on device.
"""

import sys

sys.path.insert(0, "/opt/trn_rl_repo")
import numpy as np
import ml_dtypes

P = 128
D = 48
K = 10
ALPHA = 0.1
NC = 8
NB = 98
NS = NB * P  # 12544 nodes per core
NPAD = NC * NS  # 100352
QROWS = NPAD // 4  # 25088 rows per src-quarter sub-table
ES = 128  # bf16 elems per wide table row (256 bytes)
GROUP = 7  # dst blocks per gather slab / psum group
NG = NB // GROUP  # 14

_cache = {}


def _derive(NB):
    NS = NB * P
    NPAD = NC * NS
    QROWS = NPAD // 4
    GROUP = 7 if NB % 7 == 0 else (2 if NB % 2 == 0 else 1)
    NG = NB // GROUP
    return NS, NPAD, QROWS, GROUP, NG


def _build(T_qs, R=1, NB=NB, K=K, abl=(), ncores=NC):
    NS, NPAD, QROWS, GROUP, NG = _derive(NB)
    import concourse.bass as bass
    import concourse.bacc as bacc
    import concourse.tile as tile
    from concourse import mybir
    from concourse.library_config import mlp

    f32 = mybir.dt.float32
    bf16 = mybir.dt.bfloat16
    i16 = mybir.dt.int16

    T_bsum = int(sum(T_qs))
    offB = [int(sum(T_qs[:q])) for q in range(5)]  # block-local slot offsets
    SLOTS_G = GROUP * T_bsum  # slab slots per group
    # per-(group,quarter) gather sizes and idx16 column layout
    NIDXq = [GROUP * T_qs[q] * P for q in range(4)]
    COLSq = [n // 16 for n in NIDXq]
    GCOLS = sum(COLSq)  # idx16 cols per group
    qcol0 = [int(sum(COLSq[:q])) for q in range(4)]

    nc = bacc.Bacc("TRN2", target_bir_lowering=False, debug=False, num_devices=ncores)
    feat0_d = nc.dram_tensor("feat0", [P, NB * D], f32, kind="ExternalInput")
    srcn_d = nc.dram_tensor("srcn", [P, NB], f32, kind="ExternalInput")
    dstn_d = nc.dram_tensor("dstn09", [P, NB], f32, kind="ExternalInput")
    idx16_d = nc.dram_tensor("idx16", [P, NG * GCOLS], i16, kind="ExternalInput")
    dstl_d = nc.dram_tensor("dstl", [P, NB * T_bsum], bf16, kind="ExternalInput")
    iota_d = nc.dram_tensor("iota", [P, P], bf16, kind="ExternalInput")
    out_d = nc.dram_tensor("out", [P, NB * D], f32, kind="ExternalOutput")

    with tile.TileContext(nc) as tc:
        with (
            tc.tile_pool(name="const", bufs=1) as cpool,
            tc.tile_pool(name="dram", bufs=1, space="DRAM") as dpool,
            tc.tile_pool(name="slabp", bufs=2) as slabpool,
            tc.tile_pool(name="idxp", bufs=2) as ipool,
            tc.tile_pool(name="selp", bufs=4) as spool,
            tc.tile_pool(name="eptmp", bufs=2) as wpool,
            tc.tile_pool(name="psum", bufs=2, space="PSUM") as ppool,
        ):
            nc.gpsimd.load_library(mlp)

            feat_sb = cpool.tile([P, NB * D], f32)
            nc.sync.dma_start(out=feat_sb[:], in_=feat0_d[:])
            feat0a_sb = cpool.tile([P, NB * D], f32)
            nc.scalar.mul(out=feat0a_sb[:], in_=feat_sb[:], mul=ALPHA)
            srcn_sb = cpool.tile([P, NB], f32)
            nc.sync.dma_start(out=srcn_sb[:], in_=srcn_d[:])
            dstn_sb = cpool.tile([P, NB], f32)
            nc.sync.dma_start(out=dstn_sb[:], in_=dstn_d[:])
            dstl_sb = cpool.tile([P, NB * T_bsum], bf16)
            nc.sync.dma_start(out=dstl_sb[:], in_=dstl_d[:])
            iota_sb = cpool.tile([P, P], bf16)
            nc.sync.dma_start(out=iota_sb[:], in_=iota_d[:])

            h_sb = cpool.tile([P, NB * D], bf16)
            nc.vector.tensor_tensor(
                out=h_sb[:].rearrange("p (b d) -> p b d", d=D),
                in0=feat_sb[:].rearrange("p (b d) -> p b d", d=D),
                in1=srcn_sb[:].to_broadcast([P, NB, D]),
                op=mybir.AluOpType.mult,
            )

            for r in range(R):
                for k in range(K):
                    last = (r == R - 1) and (k == K - 1)
                    h_cb = dpool.tile([NS, D], bf16, tag="hcb", bufs=2)
                    h_cf = dpool.tile(
                        [NPAD, D], bf16, addr_space="Shared", tag="hcf", bufs=2
                    )
                    h_wide = dpool.tile([NPAD, ES], bf16, tag="hwide", bufs=2)
                    nc.sync.dma_start(
                        out=h_cb[:].rearrange("(b p) d -> p b d", p=P),
                        in_=h_sb[:].rearrange("p (b d) -> p b d", d=D),
                    )
                    if "nocoll" not in abl:
                     nc.gpsimd.collective_compute(
                        "AllGather",
                        mybir.AluOpType.bypass,
                        ins=[h_cb.opt()],
                        outs=[h_cf.opt()],
                        replica_groups=[list(range(ncores))],
                    )
                    if "noexpand" not in abl:
                     for q in range(4):
                        nc.sync.dma_start(
                            out=h_wide[q * QROWS : (q + 1) * QROWS, :D],
                            in_=h_cf[q * QROWS : (q + 1) * QROWS, :],
                        )
                    for g in range(NG):
                        idxg = ipool.tile([P, GCOLS], i16, tag="idx")
                        nc.sync.dma_start(
                            out=idxg[:], in_=idx16_d[:, g * GCOLS : (g + 1) * GCOLS]
                        )
                        slab = slabpool.tile([P, SLOTS_G * ES], bf16, tag="slab")
                        if r == 0 and k == 0 and g < 2:
                            nc.vector.memset(slab[:], 0.0)
                        if "nogather" in abl:
                            nc.vector.memset(slab[:, 0:2], 0.0)
                        for q in range(4):
                            if T_qs[q] == 0 or "nogather" in abl:
                                continue
                            region = slab[
                                :, GROUP * offB[q] * ES : GROUP * offB[q + 1] * ES
                            ]
                            nc.gpsimd.dma_gather(
                                region.rearrange("p (c e) -> p c e", e=ES),
                                h_wide[q * QROWS : (q + 1) * QROWS, :],
                                idxg[:, qcol0[q] : qcol0[q] + COLSq[q]],
                                NIDXq[q],
                                NIDXq[q],
                                ES,
                                single_packet=False,
                            )
                        psum_g = ppool.tile([P, GROUP * D], f32, tag="ps")
                        if "nomm" in abl:
                            nc.vector.memset(psum_g[:], 0.0)
                        for j in range(GROUP):
                            if "nomm" in abl:
                                continue
                            b = g * GROUP + j
                            sel_sb = spool.tile([P, T_bsum * P], bf16, tag="sel")
                            if "nosel" in abl:
                                nc.vector.memset(sel_sb[:], 0.0)
                            else:
                                    nc.vector.tensor_tensor(
                                    out=sel_sb[:].rearrange("p (t w) -> p t w", t=T_bsum),
                                    in0=dstl_sb[
                                        :, b * T_bsum : (b + 1) * T_bsum
                                    ].to_broadcast([P, T_bsum, P]),
                                    in1=iota_sb[:]
                                    .unsqueeze(1)
                                    .broadcast_to([P, T_bsum, P]),
                                    op=mybir.AluOpType.is_equal,
                                )
                            mm = 0
                            for q in range(4):
                                for s in range(T_qs[q]):
                                    slot = GROUP * offB[q] + j * T_qs[q] + s
                                    selslot = offB[q] + s
                                    nc.tensor.matmul(
                                        out=psum_g[:, j * D : (j + 1) * D],
                                        lhsT=sel_sb[
                                            :, selslot * P : (selslot + 1) * P
                                        ],
                                        rhs=slab[:, slot * ES : slot * ES + D],
                                        start=(mm == 0),
                                        stop=(mm == T_bsum - 1),
                                    )
                                    mm += 1
                        gd = slice(g * GROUP * D, (g + 1) * GROUP * D)
                        tmp2 = wpool.tile([P, GROUP * D], f32, tag="tmp")
                        nc.vector.tensor_tensor(
                            out=tmp2[:].rearrange("p (b d) -> p b d", d=D),
                            in0=psum_g[:].rearrange("p (b d) -> p b d", d=D),
                            in1=dstn_sb[:, g * GROUP : (g + 1) * GROUP].to_broadcast(
                                [P, GROUP, D]
                            ),
                            op=mybir.AluOpType.mult,
                        )
                        nc.vector.tensor_tensor(
                            out=feat_sb[:, gd],
                            in0=tmp2[:],
                            in1=feat0a_sb[:, gd],
                            op=mybir.AluOpType.add,
                        )
                        if not last:
                            nc.vector.tensor_tensor(
                                out=h_sb[:, gd].rearrange("p (b d) -> p b d", d=D),
                                in0=feat_sb[:, gd].rearrange("p (b d) -> p b d", d=D),
                                in1=srcn_sb[
                                    :, g * GROUP : (g + 1) * GROUP
                                ].to_broadcast([P, GROUP, D]),
                                op=mybir.AluOpType.mult,
                            )
            nc.sync.dma_start(out=out_d[:], in_=feat_sb[:])
    nc.compile()
    return nc


def _prep(features, src, dst, NB=NB):
    NS, NPAD, QROWS, GROUP, NG = _derive(NB)
    feat = np.ascontiguousarray(np.asarray(features, np.float32))
    src = np.asarray(src).astype(np.int64)
    dst = np.asarray(dst).astype(np.int64)
    N_ = feat.shape[0]

    deg_out = np.bincount(src, minlength=N_).astype(np.float32)
    deg_in = np.bincount(dst, minlength=N_).astype(np.float32)
    srcn = 1.0 / np.sqrt(np.maximum(deg_out, 1.0))
    dstn09 = (1.0 - ALPHA) / np.sqrt(np.maximum(deg_in, 1.0))

    feat_pad = np.zeros((NPAD, D), np.float32)
    feat_pad[:N_] = feat
    srcn_pad = np.ones(NPAD, np.float32)
    srcn_pad[:N_] = srcn
    dstn_pad = np.zeros(NPAD, np.float32)
    dstn_pad[:N_] = dstn09

    # group edges by (dst block, src quarter)
    gb = dst // P  # global dst block [0, 784)
    qq = src // QROWS  # src quarter [0, 4)
    cell = gb * 4 + qq
    order = np.argsort(cell, kind="stable")
    cells = cell[order]
    srcs = src[order]
    dsts = dst[order]
    qs = qq[order]

    counts = np.bincount(cells, minlength=NC * NB * 4).reshape(NC * NB, 4)
    T_qs = tuple(
        int(x) for x in np.ceil(counts.max(axis=0) / P).astype(np.int64)
    )
    T_bsum = int(sum(T_qs))
    offB = [int(sum(T_qs[:q])) for q in range(4)]

    starts = np.zeros(NC * NB * 4 + 1, np.int64)
    starts[1:] = np.cumsum(counts.reshape(-1))
    rr = np.arange(len(dsts)) - starts[cells]
    ss = rr // P  # tile within (block, quarter)
    pp = rr % P
    cc = gb[order] // NB
    bb = gb[order] % NB

    # dstl: block-local slot order is quarter-major: slot = offB[q] + s
    offB_arr = np.array(offB, np.int64)
    slot_b = offB_arr[qs] + ss
    dstl_all = np.full((NC, P, NB * T_bsum), -1.0, np.float32)
    dstl_all[cc, pp, bb * T_bsum + slot_b] = (dsts % P).astype(np.float32)

    # gather index lists: per (core) flat list, ordered (g, q, j, s, p)
    NIDXq = [GROUP * T_qs[q] * P for q in range(4)]
    PERG = sum(NIDXq)  # 128 * GROUP * T_bsum
    base_q = np.array(
        [GROUP * offB[q] * P for q in range(4)], np.int64
    )  # within-group idx offset
    T_arr = np.array(T_qs, np.int64)
    gg_ = bb // GROUP
    jj_ = bb % GROUP
    pos = gg_ * PERG + base_q[qs] + (jj_ * T_arr[qs] + ss) * P + pp
    vals = (srcs - qs * QROWS).astype(np.int16)
    TOT = NG * PERG
    flat = np.zeros((NC, TOT), np.int16)
    flat[cc, pos] = vals

    # wrap into 16 partitions, replicate to the 8 gpsimd groups
    A = flat.reshape(NC, TOT // 16, 16)  # [c, col, j]
    B = np.swapaxes(A, 1, 2)  # [c, 16, col]
    idx16_all = np.tile(B, (1, 8, 1))  # [c, 128, col]

    feat0_all = np.ascontiguousarray(
        feat_pad.reshape(NC, NB, P, D).transpose(0, 2, 1, 3).reshape(NC, P, NB * D)
    )
    srcn_all = np.ascontiguousarray(srcn_pad.reshape(NC, NB, P).transpose(0, 2, 1))
    dstn_all = np.ascontiguousarray(dstn_pad.reshape(NC, NB, P).transpose(0, 2, 1))
    iota = np.ascontiguousarray(
        np.broadcast_to(np.arange(P, dtype=np.float32)[None, :], (P, P))
    ).astype(ml_dtypes.bfloat16)

    in_maps = [
        {
            "feat0": feat0_all[c],
            "srcn": srcn_all[c],
            "dstn09": dstn_all[c],
            "idx16": np.ascontiguousarray(idx16_all[c]),
            "dstl": np.ascontiguousarray(dstl_all[c]).astype(ml_dtypes.bfloat16),
            "iota": iota,
        }
        for c in range(NC)
    ]
    return in_maps, T_qs, N_


def _get_nc(T_qs, R=1, abl=()):
    from concourse.bass_interp import get_hw_module

    key = (T_qs, R, tuple(abl))
    if key not in _cache:
        nc = _build(T_qs, R=R, abl=abl)
        nc.m = get_hw_module(nc.m)
        _cache[key] = nc
    return _cache[key]


def kernel(features, src, dst):
    from concourse.bass_utils import run_bass_kernel_spmd

    in_maps, T_qs, N_ = _prep(features, src, dst)
    nc = _get_nc(T_qs, R=1)
    res = run_bass_kernel_spmd(nc, in_maps, core_ids=list(range(NC)))
    feat_out = np.zeros((NPAD, D), np.float32)
    for c in range(NC):
        o = res.results[c]["out"]
        feat_out[c * NS : (c + 1) * NS] = (
            o.reshape(P, NB, D).transpose(1, 0, 2).reshape(NS, D)
        )
    return feat_out[:N_].astype(np.float32)

